# revision 1
# baseline (speedup 1.0000x reference)
"""HypergraphConv (node->edge->node message passing) on 8 Trainium2 NeuronCores.

Self-contained Trainium kernel for:
    xw   = x @ W
    m_e  = (1/deg_e) * sum_{k: edge[k]=e} xw[src[k]]
    o_i  = (1/deg_i) * sum_{k: src[k]=i} m_{edge[k]} + bias
    out  = mean_i relu(o_i)                       # [128]

Sharding: nodes are split across the 8 cores (6250 each). Each core owns the
incidence entries whose src node falls in its shard; those entries drive both
the node->edge scatter (partial m, AllReduced across cores) and the
edge->node scatter (complete rows for the core's nodes).

Scatters are done as one-hot matmuls over sorted-and-padded entry streams;
gathers use the SWDGE dma_gather engine against hi/lo-bf16 row tables
(512-byte rows, ~fp32 precision via a hi+lo split accumulated in PSUM).
"""

import numpy as np
import ml_dtypes
from contextlib import ExitStack

import concourse.bacc as bacc
import concourse.bass as bass
import concourse.mybir as mybir
import concourse.tile as tile
from concourse import library_config
from concourse.bass_utils import run_bass_kernel_spmd

NCORES = 8
P = 128

# Problem sizes (overridable for small-scale testing via _configure()).
N_NODES = 50000
N_EDGES = 20000
IN_DIM = 256
OUT_DIM = 128

BF16 = mybir.dt.bfloat16
F32 = mybir.dt.float32
I16 = mybir.dt.int16

PAD_OH = 200.0  # one-hot index for padding entries: matches no iota column


def _derived():
    npc = N_NODES // NCORES
    n_node_tiles = (npc + P - 1) // P
    n_edge_tiles = (N_EDGES + P - 1) // P
    return npc, n_node_tiles, n_edge_tiles


def _wrap_idx16(idx):
    """[L] int -> [128, L//16] int16 SWDGE index layout (16-wrap, x8 replicas)."""
    a = np.asarray(idx, dtype=np.int16).reshape(-1, 16).T
    return np.ascontiguousarray(np.tile(a, (8, 1)))


def _oh_cols(oh):
    """[L] float -> [128, L//128] bf16: column c holds entries c*128..c*128+127."""
    return np.ascontiguousarray(oh.reshape(-1, P).T.astype(ml_dtypes.bfloat16))


def _bucket_entries(gidx, tid, n_tiles, chunks, pad_row):
    """Lay out (gather idx, one-hot idx) entry streams grouped by tile.

    gidx: per-entry gather row index; tid: per-entry tile id;
    chunks[t]: number of 128-entry chunks allotted to tile t (static,
    shared across cores). Returns (gather_idx[L], onehot[L]) padded streams.
    """
    order = np.argsort(tid, kind="stable")
    gidx = gidx[order]
    tid_s = tid[order]
    counts = np.bincount(tid_s, minlength=n_tiles)
    starts = np.concatenate([[0], np.cumsum(counts[:-1])])
    dest_base = np.concatenate([[0], np.cumsum(chunks[:-1])]) * P
    L = int(chunks.sum()) * P
    g_out = np.full(L, pad_row, dtype=np.int64)
    oh_out = np.full(L, PAD_OH, dtype=np.float32)
    n = gidx.shape[0]
    rank = np.arange(n, dtype=np.int64) - starts[tid_s]
    dest = dest_base[tid_s] + rank
    g_out[dest] = gidx
    # one-hot index = original row id within its tile; recomputed by caller
    return g_out, oh_out, dest, order, L


def build_kernel(chunks1, chunks2, last_nt):
    """Build the SPMD device program.

    chunks1[t]: #chunks for edge tile t (phase 1); chunks2[tt]: #chunks for
    node tile tt (phase 2); last_nt: node count of the last node tile.
    """
    npc, n_node_tiles, n_edge_tiles = _derived()
    et_rows = n_edge_tiles * P
    LA = int(np.sum(chunks1)) * P
    LB = int(np.sum(chunks2)) * P
    NCA = LA // P  # total phase-1 chunks
    NCB = LB // P

    nc = bacc.Bacc("TRN2", num_devices=NCORES)

    xT_in = nc.dram_tensor("xT", [IN_DIM, npc], F32, kind="ExternalInput")
    w_in = nc.dram_tensor("w", [IN_DIM, OUT_DIM], F32, kind="ExternalInput")
    bias_in = nc.dram_tensor("bias", [1, OUT_DIM], F32, kind="ExternalInput")
    idxA_in = nc.dram_tensor("idxA", [P, LA // 16], I16, kind="ExternalInput")
    ohA_in = nc.dram_tensor("ohA", [P, NCA], BF16, kind="ExternalInput")
    idxB_in = nc.dram_tensor("idxB", [P, LB // 16], I16, kind="ExternalInput")
    ohB_in = nc.dram_tensor("ohB", [P, NCB], BF16, kind="ExternalInput")
    out_part = nc.dram_tensor("out_part", [OUT_DIM, 1], F32, kind="ExternalOutput")

    import os
    dbg_xwhl = os.environ.get("DBG_XWHL_INPUT") == "1"
    dbg_stop = os.environ.get("DBG_STOP", "")
    if dbg_xwhl:
        xwhl = nc.dram_tensor("xwhl_in", [npc + P, 2 * OUT_DIM], BF16, kind="ExternalInput")
    else:
        xwhl = nc.dram_tensor("xwhl", [npc + P, 2 * OUT_DIM], BF16)  # zero row at npc
    m_part = nc.dram_tensor("m_part", [et_rows, OUT_DIM], F32)
    m_red = nc.dram_tensor("m_red", [et_rows, OUT_DIM], F32, addr_space="Shared")
    deg_part = nc.dram_tensor("deg_part", [P, n_edge_tiles], F32)
    deg_red = nc.dram_tensor("deg_red", [P, n_edge_tiles], F32, addr_space="Shared")
    mtab = nc.dram_tensor("mtab", [et_rows + P, 2 * OUT_DIM], BF16)

    with tile.TileContext(nc) as tc, ExitStack() as ctx:
        pin = ctx.enter_context(tc.tile_pool(name="pin", bufs=1))

        nc.gpsimd.load_library(library_config.mlp)

        # ---- persistent small tiles -------------------------------------
        iota_i = pin.tile([P, P], I16)
        iota_bf = pin.tile([P, P], BF16)
        nc.gpsimd.iota(iota_i[:], [[1, P]], channel_multiplier=0)
        nc.vector.tensor_copy(out=iota_bf[:], in_=iota_i[:])
        ones_bf = pin.tile([P, 1], BF16)
        nc.vector.memset(ones_bf[:], 1.0)
        ones_f32 = pin.tile([P, 1], F32)
        nc.vector.memset(ones_f32[:], 1.0)
        bias_bc = pin.tile([P, OUT_DIM], F32)
        nc.sync.dma_start(out=bias_bc[:], in_=bass.AP(bias_in, 0, [[0, P], [1, OUT_DIM]]))
        acc = pin.tile([P, OUT_DIM], F32)
        nc.vector.memset(acc[:], 0.0)
        deg_sb = pin.tile([P, n_edge_tiles], F32)
        # index/one-hot streams (loaded up front, used by phases 1/2)
        idxA = pin.tile([P, LA // 16], I16)
        ohA = pin.tile([P, NCA], BF16)
        idxB = pin.tile([P, LB // 16], I16)
        ohB = pin.tile([P, NCB], BF16)
        nc.sync.dma_start(out=idxA[:], in_=idxA_in[:])
        nc.sync.dma_start(out=ohA[:], in_=ohA_in[:])
        nc.sync.dma_start(out=idxB[:], in_=idxB_in[:])
        nc.sync.dma_start(out=ohB[:], in_=ohB_in[:])

        def s_build(S_tile, oh_tile, col0, k):
            """S[p, c*128+j] = (oh[p, col0+c] == j), one DVE op for k chunks."""
            s_ap = S_tile[:].rearrange("p (k j) -> p k j", k=k)
            o = oh_tile[:, col0:col0 + k]
            in0 = bass.AP(o.tensor, o.offset, [list(o.ap[0]), list(o.ap[1]), [0, P]])
            it = iota_bf[:]
            in1 = bass.AP(it.tensor, it.offset, [list(it.ap[0]), [0, k], [1, P]])
            nc.vector.tensor_tensor(out=s_ap, in0=in0, in1=in1, op=mybir.AluOpType.is_equal)

        def hilo(dst_bf, src_psum, tmp_pool, nt=P):
            """dst[:, :F] = bf16(src); dst[:, F:] = bf16(src - fp32(hi))."""
            nc.vector.tensor_copy(out=dst_bf[:nt, :OUT_DIM], in_=src_psum[:nt])
            h32 = tmp_pool.tile([P, OUT_DIM], F32, tag="h32", name="h32")
            nc.vector.tensor_copy(out=h32[:nt], in_=dst_bf[:nt, :OUT_DIM])
            nc.vector.tensor_tensor(
                out=dst_bf[:nt, OUT_DIM:], in0=src_psum[:nt], in1=h32[:nt],
                op=mybir.AluOpType.subtract)

        # ---- stage A: xw = x @ W, hi/lo table ---------------------------
        if dbg_xwhl:
            pass
        else:
          with tc.tile_pool(name="pa", bufs=1) as pa, \
             tc.tile_pool(name="pa2", bufs=3) as pa2, \
             tc.tile_pool(name="psa", bufs=2, space="PSUM") as psa:
            kh = IN_DIM // P  # k-halves
            xT_sb = [pa.tile([P, npc], F32, tag=f"xT{k}", name=f"xT{k}") for k in range(kh)]
            w_sb = [pa.tile([P, OUT_DIM], F32, tag=f"w{k}", name=f"wsb{k}") for k in range(kh)]
            for k in range(kh):
                nc.sync.dma_start(out=xT_sb[k][:], in_=xT_in[k * P:(k + 1) * P, :])
                nc.sync.dma_start(out=w_sb[k][:], in_=w_in[k * P:(k + 1) * P, :])
            zrow = pa.tile([P, 2 * OUT_DIM], BF16)
            nc.vector.memset(zrow[:], 0.0)
            nc.sync.dma_start(out=xwhl[npc:npc + P, :], in_=zrow[:])
            for i in range(0, npc, P):
                nt = min(P, npc - i)
                pxw = psa.tile([P, OUT_DIM], F32, tag="pxw")
                for k in range(kh):
                    nc.tensor.matmul(
                        out=pxw[:nt], lhsT=xT_sb[k][:, i:i + nt], rhs=w_sb[k][:],
                        start=(k == 0), stop=(k == kh - 1))
                st = pa2.tile([P, 2 * OUT_DIM], BF16, tag="xst")
                hilo(st, pxw, pa2, nt)
                nc.sync.dma_start(out=xwhl[i:i + nt, :], in_=st[:nt, :])

        # ---- stage B: phase-1 scatter (node -> edge) --------------------
        with tc.tile_pool(name="pb", bufs=3) as pb, \
             tc.tile_pool(name="psb", bufs=2, space="PSUM") as psb, \
             tc.tile_pool(name="psbd", bufs=2, space="PSUM") as psbd:
            cbase = 0
            for t in range(n_edge_tiles):
                kt = int(chunks1[t])
                ni = kt * P
                G = pb.tile([P, kt, 2 * OUT_DIM], BF16, tag="G")
                for g0 in range(0, kt, 64):
                    gk = min(64, kt - g0)
                    nc.gpsimd.dma_gather(
                        G[:, g0:g0 + gk, :], xwhl[:, :],
                        idxA[:, (cbase + g0) * 8:(cbase + g0 + gk) * 8],
                        gk * P, gk * P, 2 * OUT_DIM, single_packet=False)
                S = pb.tile([P, kt * P], BF16, tag="S")
                s_build(S, ohA, cbase, kt)
                pm = psb.tile([P, 2 * OUT_DIM], F32, tag="pm")
                pdeg = psbd.tile([P, 1], F32, tag="pdeg")
                for c in range(kt):
                    nc.tensor.matmul(
                        out=pm[:], lhsT=S[:, c * P:(c + 1) * P], rhs=G[:, c, :],
                        start=(c == 0), stop=(c == kt - 1), skip_group_check=True)
                    nc.tensor.matmul(
                        out=pdeg[:], lhsT=S[:, c * P:(c + 1) * P], rhs=ones_bf[:],
                        start=(c == 0), stop=(c == kt - 1), skip_group_check=True)
                mt = pb.tile([P, OUT_DIM], F32, tag="mt")
                nc.vector.tensor_copy(out=mt[:], in_=pm[:, :OUT_DIM])
                nc.vector.tensor_tensor(
                    out=mt[:], in0=mt[:], in1=pm[:, OUT_DIM:], op=mybir.AluOpType.add)
                nc.vector.tensor_copy(out=deg_sb[:, t:t + 1], in_=pdeg[:])
                nc.sync.dma_start(out=m_part[t * P:(t + 1) * P, :], in_=mt[:])
                cbase += kt
            nc.sync.dma_start(out=deg_part[:], in_=deg_sb[:])

        if dbg_stop == "B":
            with tc.tile_pool(name="pz", bufs=1) as pz:
                oc = pz.tile([P, 1], F32)
                nc.vector.tensor_copy(out=oc[:], in_=deg_sb[:, 0:1])
                nc.sync.dma_start(out=out_part[:, :], in_=oc[:])

        # ---- stage C: AllReduce + m' table ------------------------------
        dbg_no_cc = os.environ.get("DBG_NO_CC") == "1"
        if dbg_stop != "B":
          if dbg_no_cc:
            nc.sync.dma_start(out=m_red[:, :], in_=m_part[:, :])
            nc.sync.dma_start(out=deg_red[:, :], in_=deg_part[:, :])
          else:
            nc.gpsimd.collective_compute(
              "AllReduce", mybir.AluOpType.add, replica_groups=[list(range(NCORES))],
              ins=[m_part[:, :]], outs=[m_red[:, :]])
            nc.gpsimd.collective_compute(
              "AllReduce", mybir.AluOpType.add, replica_groups=[list(range(NCORES))],
              ins=[deg_part[:, :]], outs=[deg_red[:, :]])

          with tc.tile_pool(name="pc", bufs=3) as pc:
              dga = pc.tile([P, n_edge_tiles], F32, tag="dga")
              nc.sync.dma_start(out=dga[:], in_=deg_red[:])
              binv = pin.tile([P, n_edge_tiles], F32)
              nc.vector.tensor_scalar(
                  out=binv[:], in0=dga[:], scalar1=1.0, scalar2=None,
                  op0=mybir.AluOpType.max)
              nc.vector.reciprocal(out=binv[:], in_=binv[:])
              zrow2 = pc.tile([P, 2 * OUT_DIM], BF16, tag="zr2")
              nc.vector.memset(zrow2[:], 0.0)
              nc.sync.dma_start(out=mtab[et_rows:et_rows + P, :], in_=zrow2[:])
              for t in range(n_edge_tiles):
                  mt = pc.tile([P, OUT_DIM], F32, tag="mtc")
                  nc.sync.dma_start(out=mt[:], in_=m_red[t * P:(t + 1) * P, :])
                  nc.vector.tensor_scalar(
                      out=mt[:], in0=mt[:], scalar1=binv[:, t:t + 1], scalar2=None,
                      op0=mybir.AluOpType.mult)
                  st = pc.tile([P, 2 * OUT_DIM], BF16, tag="mst")
                  hilo(st, mt, pc)
                  nc.sync.dma_start(out=mtab[t * P:(t + 1) * P, :], in_=st[:])

        if dbg_stop == "C":
            with tc.tile_pool(name="pz2", bufs=1) as pz2:
                oc2 = pz2.tile([P, 1], F32)
                nc.vector.tensor_copy(out=oc2[:], in_=binv[:, 0:1])
                nc.sync.dma_start(out=out_part[:, :], in_=oc2[:])

        # ---- stage D: phase-2 scatter (edge -> node) + post -------------
        run_d = dbg_stop not in ("B", "C")
        run_e = run_d and dbg_stop != "D"
        d_lvl = int(os.environ.get("DBG_D_LVL", "4"))

        if run_d:
          with tc.tile_pool(name="pd", bufs=3) as pd, \
             tc.tile_pool(name="psd", bufs=2, space="PSUM") as psd, \
             tc.tile_pool(name="psdd", bufs=2, space="PSUM") as psdd:
            cbase = 0
            for tt in range(n_node_tiles):
                kt = int(chunks2[tt])
                ni = kt * P
                nt = last_nt if tt == n_node_tiles - 1 else P
                G = pd.tile([P, kt, 2 * OUT_DIM], BF16, tag="G2")
                for g0 in range(0, kt, 64):
                    gk = min(64, kt - g0)
                    nc.gpsimd.dma_gather(
                        G[:, g0:g0 + gk, :], mtab[:, :],
                        idxB[:, (cbase + g0) * 8:(cbase + g0 + gk) * 8],
                        gk * P, gk * P, 2 * OUT_DIM, single_packet=False)
                if d_lvl <= 1:
                    nc.vector.tensor_copy(out=acc[:, tt:tt + 1], in_=G[:, 0, 0:1])
                    cbase += kt
                    continue
                S = pd.tile([P, kt * P], BF16, tag="S2")
                s_build(S, ohB, cbase, kt)
                if d_lvl <= 2:
                    nc.vector.tensor_copy(out=acc[:, tt:tt + 1], in_=S[:, 0:1])
                    cbase += kt
                    continue
                po = psd.tile([P, 2 * OUT_DIM], F32, tag="po")
                pdeg = psdd.tile([P, 1], F32, tag="pdeg2")
                for c in range(kt):
                    nc.tensor.matmul(
                        out=po[:], lhsT=S[:, c * P:(c + 1) * P], rhs=G[:, c, :],
                        start=(c == 0), stop=(c == kt - 1), skip_group_check=True)
                    nc.tensor.matmul(
                        out=pdeg[:], lhsT=S[:, c * P:(c + 1) * P], rhs=ones_bf[:],
                        start=(c == 0), stop=(c == kt - 1), skip_group_check=True)
                if d_lvl <= 3:
                    nc.vector.tensor_copy(out=acc[:, tt:tt + 1], in_=po[:, 0:1])
                    cbase += kt
                    continue
                dinv = pd.tile([P, 1], F32, tag="dinv")
                nc.vector.tensor_scalar(
                    out=dinv[:], in0=pdeg[:], scalar1=1.0, scalar2=None,
                    op0=mybir.AluOpType.max)
                nc.vector.reciprocal(out=dinv[:], in_=dinv[:])
                ot = pd.tile([P, OUT_DIM], F32, tag="ot")
                nc.vector.tensor_copy(out=ot[:nt], in_=po[:nt, :OUT_DIM])
                nc.vector.tensor_tensor(
                    out=ot[:nt], in0=ot[:nt], in1=po[:nt, OUT_DIM:],
                    op=mybir.AluOpType.add)
                nc.vector.tensor_scalar(
                    out=ot[:nt], in0=ot[:nt], scalar1=dinv[:nt, :1], scalar2=None,
                    op0=mybir.AluOpType.mult)
                nc.vector.tensor_tensor(
                    out=ot[:nt], in0=ot[:nt], in1=bias_bc[:nt], op=mybir.AluOpType.add)
                nc.vector.tensor_scalar(
                    out=ot[:nt], in0=ot[:nt], scalar1=0.0, scalar2=None,
                    op0=mybir.AluOpType.max)
                nc.vector.tensor_tensor(
                    out=acc[:nt], in0=acc[:nt], in1=ot[:nt], op=mybir.AluOpType.add)
                cbase += kt
            if dbg_stop == "D":
                oc3 = pd.tile([P, 1], F32, tag="oc3", name="oc3")
                nc.vector.tensor_copy(out=oc3[:], in_=acc[:, 0:1])
                nc.sync.dma_start(out=out_part[:, :], in_=oc3[:])

        # ---- stage E: column sum over nodes -> [OUT_DIM, 1] -------------
        if run_e:
          with tc.tile_pool(name="pe", bufs=1) as pe, \
             tc.tile_pool(name="pse", bufs=1, space="PSUM") as pse:
            pcol = pse.tile([P, 1], F32)
            nc.tensor.matmul(out=pcol[:OUT_DIM], lhsT=acc[:], rhs=ones_f32[:],
                             start=True, stop=True)
            ocol = pe.tile([P, 1], F32)
            nc.vector.tensor_copy(out=ocol[:OUT_DIM], in_=pcol[:OUT_DIM])
            nc.sync.dma_start(out=out_part[:, :], in_=ocol[:OUT_DIM])

    nc.compile()
    return nc


def prepare_inputs(x, w, bias, hyperedge_index):
    """Host-side sharding: split entries by src-node shard, sort/pad both
    phase streams, compute the static chunk structure shared by all cores."""
    npc, n_node_tiles, n_edge_tiles = _derived()
    src = np.asarray(hyperedge_index[0], dtype=np.int64)
    edge = np.asarray(hyperedge_index[1], dtype=np.int64)

    core_of = src // npc
    per_core = []
    for c in range(NCORES):
        sel = core_of == c
        per_core.append((src[sel] - c * npc, edge[sel]))

    # static chunk structure = max over cores, per tile
    cnt1 = np.zeros((NCORES, n_edge_tiles), np.int64)
    cnt2 = np.zeros((NCORES, n_node_tiles), np.int64)
    for c, (s_loc, e_glob) in enumerate(per_core):
        cnt1[c] = np.bincount(e_glob // P, minlength=n_edge_tiles)
        cnt2[c] = np.bincount(s_loc // P, minlength=n_node_tiles)
    chunks1 = np.maximum(1, -(-cnt1.max(axis=0) // P))
    chunks2 = np.maximum(1, -(-cnt2.max(axis=0) // P))

    in_maps = []
    for c, (s_loc, e_glob) in enumerate(per_core):
        # phase 1: group by edge tile; gather xwhl[s_loc], one-hot = edge%P
        t1 = e_glob // P
        g1, oh1, dest1, order1, LA = _bucket_entries(s_loc, t1, n_edge_tiles, chunks1, npc)
        oh1[dest1] = (e_glob % P)[order1].astype(np.float32)
        # phase 2: group by node tile; gather mtab[e_glob], one-hot = s_loc%P
        t2 = s_loc // P
        g2, oh2, dest2, order2, LB = _bucket_entries(
            e_glob, t2, n_node_tiles, chunks2, n_edge_tiles * P)
        oh2[dest2] = (s_loc % P)[order2].astype(np.float32)

        xT = np.ascontiguousarray(x[c * npc:(c + 1) * npc].T.astype(np.float32))
        in_maps.append({
            "xT": xT,
            "w": np.ascontiguousarray(w.astype(np.float32)),
            "bias": np.ascontiguousarray(bias.astype(np.float32)).reshape(1, -1),
            "idxA": _wrap_idx16(g1),
            "ohA": _oh_cols(oh1),
            "idxB": _wrap_idx16(g2),
            "ohB": _oh_cols(oh2),
        })

    last_nt = npc - (n_node_tiles - 1) * P
    return in_maps, chunks1, chunks2, last_nt


def kernel(x_node_features, lin_weight, bias, hyperedge_index):
    in_maps, chunks1, chunks2, last_nt = prepare_inputs(
        x_node_features, lin_weight, bias, hyperedge_index)
    nc = build_kernel(chunks1, chunks2, last_nt)
    res = run_bass_kernel_spmd(nc, in_maps, list(range(NCORES)))
    total = np.zeros(OUT_DIM, np.float64)
    for c in range(NCORES):
        total += res.results[c]["out_part"][:, 0].astype(np.float64)
    return (total / N_NODES).astype(np.float32)



# revision 9
# speedup vs baseline: 1.7675x; 1.7675x over previous
"""HypergraphConv (node->edge->node message passing) on 8 Trainium2 NeuronCores.

Self-contained Trainium kernel for:
    xw   = x @ W
    m_e  = (1/deg_e) * sum_{k: edge[k]=e} xw[src[k]]
    o_i  = (1/deg_i) * sum_{k: src[k]=i} m_{edge[k]} + bias
    out  = mean_i relu(o_i)                       # [128]

Sharding: nodes are split across the 8 cores (6250 each). Each core owns the
incidence entries whose src node falls in its shard; those entries drive both
the node->edge scatter (partial m, ReduceScattered + AllGathered across cores)
and the edge->node scatter (complete rows for the core's nodes).

Scatters are one-hot matmuls over sorted-and-padded entry streams grouped into
64-wide edge/node tiles; gathers use the SWDGE dma_gather engine against bf16
row tables (256-byte rows). Degrees (exact integer reciprocals) are computed
host-side and shipped as small inputs. The phase-2 matmul is transposed
(out = G^T-style [feat, node]) so bias+ReLU fuse into one Activation-engine op
writing straight into the accumulation buffer.
"""

import os
import numpy as np
import ml_dtypes
from contextlib import ExitStack

import concourse.bacc as bacc
import concourse.bass as bass
import concourse.mybir as mybir
import concourse.tile as tile
from concourse import library_config
from concourse.bass_utils import run_bass_kernel_spmd

NCORES = 8
P = 128

N_NODES = 50000
N_EDGES = 20000
IN_DIM = 256
OUT_DIM = 128

ETW = 64           # edge-tile (group) width for phase-1 scatter
NTW = 64           # node-tile width for phase-2 scatter
SUPER = 48         # max 128-entry chunks per dma_gather call

BF16 = mybir.dt.bfloat16
F32 = mybir.dt.float32
I16 = mybir.dt.int16

PAD_OH = 200.0  # one-hot index for padding entries: matches no iota column


def _derived():
    npc = N_NODES // NCORES
    n_node_groups = (npc + NTW - 1) // NTW
    quantum = NCORES * P
    erows = -(-N_EDGES // quantum) * quantum  # RS-shardable, mult of 128
    n_edge_groups = erows // ETW
    return npc, n_node_groups, n_edge_groups, erows, erows // NCORES


def _wrap_idx16(idx):
    """[L] int -> [128, L//16] int16 SWDGE index layout (16-wrap, x8 replicas)."""
    a = np.asarray(idx, dtype=np.int16).reshape(-1, 16).T
    return np.ascontiguousarray(np.tile(a, (8, 1)))


def _oh_cols(oh):
    """[L] float -> [128, L//128] bf16: column c holds entries c*128..c*128+127."""
    return np.ascontiguousarray(oh.reshape(-1, P).T.astype(ml_dtypes.bfloat16))


def _bucket_entries(gidx, ohval, tid, n_tiles, chunks):
    """Lay out (gather idx, one-hot) entry streams grouped by tile.

    chunks[t]: number of 128-entry chunks allotted to tile t (static, shared
    across cores). Pad gather idx = 0 (contribution killed by the all-zero
    one-hot row). Returns (gather_idx[L], onehot[L]).
    """
    order = np.argsort(tid, kind="stable")
    gidx = gidx[order]
    ohval = ohval[order]
    tid_s = tid[order]
    counts = np.bincount(tid_s, minlength=n_tiles)
    starts = np.concatenate([[0], np.cumsum(counts[:-1])])
    dest_base = np.concatenate([[0], np.cumsum(chunks[:-1])]) * P
    L = int(chunks.sum()) * P
    g_out = np.zeros(L, dtype=np.int64)
    oh_out = np.full(L, PAD_OH, dtype=np.float32)
    n = gidx.shape[0]
    rank = np.arange(n, dtype=np.int64) - starts[tid_s]
    dest = dest_base[tid_s] + rank
    g_out[dest] = gidx
    oh_out[dest] = ohval
    return g_out, oh_out


def build_kernel(chunks1, chunks2):
    """Build the SPMD device program.

    chunks1[g]: #chunks for edge group g (phase 1, may be 0);
    chunks2[gg]: #chunks for node group gg (phase 2, >= 1).
    """
    npc, n_node_groups, n_edge_groups, EROWS, SHARD = _derived()
    NROWS = n_node_groups * NTW
    LA = int(np.sum(chunks1)) * P
    LB = int(np.sum(chunks2)) * P
    NCA = LA // P
    NCB = LB // P
    REPS = int(os.environ.get("KREPS", "1"))

    nc = bacc.Bacc("TRN2", num_devices=NCORES)

    xT_in = nc.dram_tensor("xT", [IN_DIM, npc], F32, kind="ExternalInput")
    w_in = nc.dram_tensor("w", [IN_DIM, OUT_DIM], F32, kind="ExternalInput")
    biasT_in = nc.dram_tensor("biasT", [OUT_DIM, 1], F32, kind="ExternalInput")
    dinv_in = nc.dram_tensor("dinv", [1, NROWS], F32, kind="ExternalInput")
    binv_in = nc.dram_tensor("binv", [P, SHARD // P], F32, kind="ExternalInput")
    idxA_in = nc.dram_tensor("idxA", [P, LA // 16], I16, kind="ExternalInput")
    ohA_in = nc.dram_tensor("ohA", [P, NCA], BF16, kind="ExternalInput")
    idxB_in = nc.dram_tensor("idxB", [P, LB // 16], I16, kind="ExternalInput")
    ohB_in = nc.dram_tensor("ohB", [P, NCB], BF16, kind="ExternalInput")
    out_part = nc.dram_tensor("out_part", [OUT_DIM, 1], F32, kind="ExternalOutput")

    xwhl = nc.dram_tensor("xwhl", [npc, OUT_DIM], BF16)
    m_part = nc.dram_tensor("m_part", [EROWS, OUT_DIM], BF16)
    m_shard = nc.dram_tensor("m_shard", [SHARD, OUT_DIM], BF16)
    mtab_sh = nc.dram_tensor("mtab_sh", [SHARD, OUT_DIM], BF16)
    mtab = nc.dram_tensor("mtab", [EROWS, OUT_DIM], BF16, addr_space="Shared")

    # phase-1 super-groups: consecutive edge groups gathered in one call
    def make_supers(chunks):
        supers = []  # (chunk_offset, [(tile_idx, kt, local_chunk_off)])
        cur, ck, coff = [], 0, 0
        base = 0
        for t, k in enumerate(chunks):
            k = int(k)
            if k == 0:
                continue
            if ck + k > SUPER and cur:
                supers.append((coff, cur))
                coff += ck
                cur, ck = [], 0
            cur.append((t, k, ck))
            ck += k
        if cur:
            supers.append((coff, cur))
        return supers

    supers1 = make_supers(chunks1)
    supers2 = make_supers(chunks2)
    empty1 = [t for t, k in enumerate(chunks1) if int(k) == 0]

    with tile.TileContext(nc) as tc, ExitStack() as ctx:
        pin = ctx.enter_context(tc.tile_pool(name="pin", bufs=1))

        nc.gpsimd.load_library(library_config.mlp)

        # ---- persistent small tiles (once, outside reps) ----------------
        iota_i = pin.tile([P, P], I16)
        iota_bf = pin.tile([P, P], BF16)
        nc.gpsimd.iota(iota_i[:], [[1, P]], channel_multiplier=0)
        nc.vector.tensor_copy(out=iota_bf[:], in_=iota_i[:])

        def s_build(S_tile, oh_tile, col0, k, w):
            """S[p, c*w+j] = (oh[p, col0+c] == j), one DVE op for k chunks."""
            s_ap = S_tile[:].rearrange("p (k j) -> p k j", k=k)
            o = oh_tile[:, col0:col0 + k]
            in0 = bass.AP(o.tensor, o.offset, [list(o.ap[0]), list(o.ap[1]), [0, w]])
            it = iota_bf[:]
            in1 = bass.AP(it.tensor, it.offset, [list(it.ap[0]), [0, k], [1, w]])
            nc.vector.tensor_tensor(out=s_ap, in0=in0, in1=in1, op=mybir.AluOpType.is_equal)

        for rep in range(REPS):
          with tc.tile_pool(name=f"prep{rep}", bufs=1) as pr:
            # streams + per-rep persistent tiles
            idxA = pr.tile([P, LA // 16], I16, name="idxA")
            ohA = pr.tile([P, NCA], BF16, name="ohA")
            idxB = pr.tile([P, LB // 16], I16, name="idxB")
            ohB = pr.tile([P, NCB], BF16, name="ohB")
            nc.sync.dma_start(out=idxA[:], in_=idxA_in[:])
            nc.sync.dma_start(out=ohA[:], in_=ohA_in[:])
            nc.sync.dma_start(out=idxB[:], in_=idxB_in[:])
            nc.sync.dma_start(out=ohB[:], in_=ohB_in[:])
            bias_sb = pr.tile([P, 1], F32, name="biasT")
            nc.sync.dma_start(out=bias_sb[:OUT_DIM], in_=biasT_in[:])
            dinv_bc = pr.tile([P, NROWS], F32, name="dinvbc")
            nc.sync.dma_start(
                out=dinv_bc[:], in_=bass.AP(dinv_in, 0, [[0, P], [1, NROWS]]))
            binv_sb = pr.tile([P, SHARD // P], F32, name="binv")
            nc.sync.dma_start(out=binv_sb[:], in_=binv_in[:])
            accT = pr.tile([P, NROWS], F32, name="accT")

            # ---- stage A: xw = x @ W -> bf16 row table ------------------
            with tc.tile_pool(name=f"pa{rep}", bufs=1) as pa, \
                 tc.tile_pool(name=f"pa2{rep}", bufs=3) as pa2, \
                 tc.tile_pool(name=f"psa{rep}", bufs=2, space="PSUM") as psa:
                kh = IN_DIM // P
                xT_sb = [pa.tile([P, npc], F32, name=f"xT{k}") for k in range(kh)]
                w_sb = [pa.tile([P, OUT_DIM], F32, name=f"wsb{k}") for k in range(kh)]
                for k in range(kh):
                    nc.sync.dma_start(out=xT_sb[k][:], in_=xT_in[k * P:(k + 1) * P, :])
                    nc.sync.dma_start(out=w_sb[k][:], in_=w_in[k * P:(k + 1) * P, :])
                for i in range(0, npc, P):
                    nt = min(P, npc - i)
                    pxw = psa.tile([P, OUT_DIM], F32, tag="pxw")
                    for k in range(kh):
                        nc.tensor.matmul(
                            out=pxw[:nt], lhsT=xT_sb[k][:, i:i + nt], rhs=w_sb[k][:],
                            start=(k == 0), stop=(k == kh - 1))
                    xst = pa2.tile([P, OUT_DIM], BF16, tag="xst")
                    nc.scalar.activation(
                        out=xst[:nt], in_=pxw[:nt],
                        func=mybir.ActivationFunctionType.Copy)
                    nc.sync.dma_start(out=xwhl[i:i + nt, :], in_=xst[:nt, :])

            # ---- stage B: phase-1 scatter (node -> edge) ----------------
            with tc.tile_pool(name=f"pb{rep}", bufs=3) as pb, \
                 tc.tile_pool(name=f"psb{rep}", bufs=2, space="PSUM") as psb:
                zrow = pb.tile([P, OUT_DIM], BF16, tag="zrow", name="zrow")
                nc.vector.memset(zrow[:], 0.0)
                for t in empty1:
                    nc.sync.dma_start(
                        out=m_part[t * ETW:(t + 1) * ETW, :], in_=zrow[:ETW, :])
                for coff, groups in supers1:
                    ck_tot = sum(k for _, k, _ in groups)
                    G = pb.tile([P, ck_tot, OUT_DIM], BF16, tag="G")
                    nc.gpsimd.dma_gather(
                        G[:, :, :], xwhl[:, :],
                        idxA[:, coff * 8:(coff + ck_tot) * 8],
                        ck_tot * P, ck_tot * P, OUT_DIM, single_packet=False)
                    for t, kt, loc in groups:
                        S = pb.tile([P, kt * ETW], BF16, tag="S")
                        s_build(S, ohA, coff + loc, kt, ETW)
                        pm = psb.tile([P, OUT_DIM], F32, tag="pm")
                        for c in range(kt):
                            nc.tensor.matmul(
                                out=pm[:ETW], lhsT=S[:, c * ETW:(c + 1) * ETW],
                                rhs=G[:, loc + c, :],
                                start=(c == 0), stop=(c == kt - 1),
                                skip_group_check=True)
                        mt = pb.tile([P, OUT_DIM], BF16, tag="mt")
                        nc.scalar.activation(
                            out=mt[:ETW], in_=pm[:ETW],
                            func=mybir.ActivationFunctionType.Copy)
                        nc.sync.dma_start(
                            out=m_part[t * ETW:(t + 1) * ETW, :], in_=mt[:ETW, :])

            # ---- stage C: ReduceScatter -> scale -> AllGather -----------
            no_cc = os.environ.get("DBG_NO_CC") == "1"  # TimelineSim can't
            if no_cc:                                   # model collectives
                nc.sync.dma_start(out=m_shard[:, :], in_=m_part[:SHARD, :])
            else:
                nc.gpsimd.collective_compute(
                    "ReduceScatter", mybir.AluOpType.add,
                    replica_groups=[list(range(NCORES))],
                    ins=[m_part[:, :]], outs=[m_shard[:, :]])
            with tc.tile_pool(name=f"pc{rep}", bufs=3) as pc:
                for t in range(SHARD // P):
                    ms = pc.tile([P, OUT_DIM], BF16, tag="ms")
                    nc.sync.dma_start(out=ms[:], in_=m_shard[t * P:(t + 1) * P, :])
                    st = pc.tile([P, OUT_DIM], BF16, tag="st")
                    nc.vector.tensor_scalar(
                        out=st[:], in0=ms[:], scalar1=binv_sb[:, t:t + 1],
                        scalar2=None, op0=mybir.AluOpType.mult)
                    nc.sync.dma_start(out=mtab_sh[t * P:(t + 1) * P, :], in_=st[:])
            if no_cc:
                for cc in range(NCORES):
                    nc.sync.dma_start(
                        out=mtab[cc * SHARD:(cc + 1) * SHARD, :],
                        in_=mtab_sh[:, :])
            else:
                nc.gpsimd.collective_compute(
                    "AllGather", mybir.AluOpType.bypass,
                    replica_groups=[list(range(NCORES))],
                    ins=[mtab_sh[:, :]], outs=[mtab[:, :]])

            # ---- stage D: phase-2 scatter (edge -> node), transposed ----
            with tc.tile_pool(name=f"pd{rep}", bufs=3) as pd, \
                 tc.tile_pool(name=f"psd{rep}", bufs=2, space="PSUM") as psd:
                for coff, groups in supers2:
                    ck_tot = sum(k for _, k, _ in groups)
                    G2 = pd.tile([P, ck_tot, OUT_DIM], BF16, tag="G2")
                    nc.gpsimd.dma_gather(
                        G2[:, :, :], mtab[:, :],
                        idxB[:, coff * 8:(coff + ck_tot) * 8],
                        ck_tot * P, ck_tot * P, OUT_DIM, single_packet=False)
                    for gg, kt, loc in groups:
                        S2 = pd.tile([P, kt * NTW], BF16, tag="S2")
                        s_build(S2, ohB, coff + loc, kt, NTW)
                        poT = psd.tile([P, NTW], F32, tag="poT")
                        for c in range(kt):
                            nc.tensor.matmul(
                                out=poT[:], lhsT=G2[:, loc + c, :],
                                rhs=S2[:, c * NTW:(c + 1) * NTW],
                                start=(c == 0), stop=(c == kt - 1),
                                skip_group_check=True)
                        ot = pd.tile([P, NTW], F32, tag="ot")
                        nc.vector.tensor_tensor(
                            out=ot[:], in0=poT[:],
                            in1=dinv_bc[:, gg * NTW:(gg + 1) * NTW],
                            op=mybir.AluOpType.mult)
                        nc.scalar.activation(
                            out=accT[:, gg * NTW:(gg + 1) * NTW], in_=ot[:],
                            func=mybir.ActivationFunctionType.Relu,
                            bias=bias_sb[:, 0:1])
                # zero phantom-node columns (beyond npc) before the reduce
                if NROWS > npc:
                    nc.vector.memset(accT[:, npc:NROWS], 0.0)

            # ---- stage E: row-sum over all node columns -> [OUT_DIM, 1] -
            with tc.tile_pool(name=f"pe{rep}", bufs=1) as pe:
                osum = pe.tile([P, 1], F32)
                nc.vector.tensor_reduce(
                    out=osum[:], in_=accT[:], axis=mybir.AxisListType.X,
                    op=mybir.AluOpType.add)
                nc.sync.dma_start(out=out_part[:, :], in_=osum[:OUT_DIM])

    nc.compile()
    return nc


def prepare_inputs(x, w, bias, hyperedge_index):
    """Host-side sharding: split entries by src-node shard, bucket/pad both
    phase streams, compute degrees + static chunk structure (shared by all
    cores)."""
    npc, n_node_groups, n_edge_groups, EROWS, SHARD = _derived()
    NROWS = n_node_groups * NTW
    src = np.asarray(hyperedge_index[0], dtype=np.int64)
    edge = np.asarray(hyperedge_index[1], dtype=np.int64)

    # exact degree reciprocals (host)
    deg_e = np.bincount(edge, minlength=N_EDGES).astype(np.float64)
    b_inv_full = np.zeros(EROWS, np.float32)
    nzmask = deg_e > 0
    b_inv_full[:N_EDGES][nzmask] = (1.0 / deg_e[nzmask]).astype(np.float32)

    core_of = src // npc
    per_core = []
    for c in range(NCORES):
        sel = core_of == c
        per_core.append((src[sel] - c * npc, edge[sel]))

    cnt1 = np.zeros((NCORES, n_edge_groups), np.int64)
    cnt2 = np.zeros((NCORES, n_node_groups), np.int64)
    for c, (s_loc, e_glob) in enumerate(per_core):
        cnt1[c] = np.bincount(e_glob // ETW, minlength=n_edge_groups)
        cnt2[c] = np.bincount(s_loc // NTW, minlength=n_node_groups)
    chunks1 = -(-cnt1.max(axis=0) // P)                 # may be 0
    chunks2 = np.maximum(1, -(-cnt2.max(axis=0) // P))  # >= 1 (bias/relu rows)

    in_maps = []
    for c, (s_loc, e_glob) in enumerate(per_core):
        g1, oh1 = _bucket_entries(
            s_loc, (e_glob % ETW).astype(np.float32), e_glob // ETW,
            n_edge_groups, chunks1)
        g2, oh2 = _bucket_entries(
            e_glob, (s_loc % NTW).astype(np.float32), s_loc // NTW,
            n_node_groups, chunks2)

        deg_n = np.bincount(s_loc, minlength=npc).astype(np.float64)
        d_inv = np.zeros(NROWS, np.float32)
        nz = deg_n > 0
        d_inv[:npc][nz] = (1.0 / deg_n[nz]).astype(np.float32)

        binv_shard = np.ascontiguousarray(
            b_inv_full[c * SHARD:(c + 1) * SHARD].reshape(SHARD // P, P).T)

        xT = np.ascontiguousarray(x[c * npc:(c + 1) * npc].T.astype(np.float32))
        in_maps.append({
            "xT": xT,
            "w": np.ascontiguousarray(w.astype(np.float32)),
            "biasT": np.ascontiguousarray(bias.astype(np.float32)).reshape(-1, 1),
            "dinv": d_inv.reshape(1, -1),
            "binv": binv_shard,
            "idxA": _wrap_idx16(g1),
            "ohA": _oh_cols(oh1),
            "idxB": _wrap_idx16(g2),
            "ohB": _oh_cols(oh2),
        })

    return in_maps, chunks1, chunks2


def kernel(x_node_features, lin_weight, bias, hyperedge_index):
    in_maps, chunks1, chunks2 = prepare_inputs(
        x_node_features, lin_weight, bias, hyperedge_index)
    nc = build_kernel(chunks1, chunks2)
    res = run_bass_kernel_spmd(nc, in_maps, list(range(NCORES)))
    total = np.zeros(OUT_DIM, np.float64)
    for c in range(NCORES):
        total += res.results[c]["out_part"][:, 0].astype(np.float64)
    return (total / N_NODES).astype(np.float32)


# revision 19
# speedup vs baseline: 1.7936x; 1.0148x over previous
"""HypergraphConv (node->edge->node message passing) on 8 Trainium2 NeuronCores.

Self-contained Trainium kernel for:
    xw   = x @ W
    m_e  = (1/deg_e) * sum_{k: edge[k]=e} xw[src[k]]
    o_i  = (1/deg_i) * sum_{k: src[k]=i} m_{edge[k]} + bias
    out  = mean_i relu(o_i)                       # [128]

Sharding: nodes are split across the 8 cores (6250 each). Each core owns the
incidence entries whose src node falls in its shard; those entries drive both
the node->edge scatter (partial m, ReduceScattered + AllGathered across cores)
and the edge->node scatter (complete rows for the core's nodes).

Scatters are one-hot matmuls over sorted-and-padded entry streams grouped into
64-wide edge/node tiles; gathers use the SWDGE dma_gather engine against bf16
row tables (256-byte rows). Degrees (exact integer reciprocals) are computed
host-side and shipped as small inputs. The phase-2 matmul is transposed
(out = G^T-style [feat, node]) so bias+ReLU fuse into one Activation-engine op
writing straight into the accumulation buffer.
"""

import os
import numpy as np
import ml_dtypes
from contextlib import ExitStack

import concourse.bacc as bacc
import concourse.bass as bass
import concourse.mybir as mybir
import concourse.tile as tile
from concourse import library_config
from concourse.bass_utils import run_bass_kernel_spmd

NCORES = 8
P = 128

N_NODES = 50000
N_EDGES = 20000
IN_DIM = 256
OUT_DIM = 128

ETW = 128          # edge-tile (group) width for phase-1 scatter
NTW = 64           # node-tile width for phase-2 scatter
SUPER = 48         # max 128-entry chunks per dma_gather call

BF16 = mybir.dt.bfloat16
F32 = mybir.dt.float32
I16 = mybir.dt.int16

PAD_OH = 200.0  # one-hot index for padding entries: matches no iota column


def _derived():
    npc = N_NODES // NCORES
    n_node_groups = (npc + NTW - 1) // NTW
    quantum = NCORES * P
    erows = -(-N_EDGES // quantum) * quantum  # RS-shardable, mult of 128
    n_edge_groups = erows // ETW
    return npc, n_node_groups, n_edge_groups, erows, erows // NCORES


def _wrap_idx16(idx):
    """[L] int -> [128, L//16] int16 SWDGE index layout (16-wrap, x8 replicas)."""
    a = np.asarray(idx, dtype=np.int16).reshape(-1, 16).T
    return np.ascontiguousarray(np.tile(a, (8, 1)))


def _oh_cols(oh):
    """[L] float -> [128, L//128] bf16: column c holds entries c*128..c*128+127."""
    return np.ascontiguousarray(oh.reshape(-1, P).T.astype(ml_dtypes.bfloat16))


def _bucket_entries(gidx, ohval, tid, n_tiles, chunks):
    """Lay out (gather idx, one-hot) entry streams grouped by tile.

    chunks[t]: number of 128-entry chunks allotted to tile t (static, shared
    across cores). Pad gather idx = 0 (contribution killed by the all-zero
    one-hot row). Returns (gather_idx[L], onehot[L]).
    """
    # sort by (tile, gather idx): idx-sorted slots give the SWDGE's 16-idx
    # descriptors HBM row-buffer locality; slot order is free (one-hot maps it)
    order = np.lexsort((gidx, tid))
    gidx = gidx[order]
    ohval = ohval[order]
    tid_s = tid[order]
    counts = np.bincount(tid_s, minlength=n_tiles)
    starts = np.concatenate([[0], np.cumsum(counts[:-1])])
    dest_base = np.concatenate([[0], np.cumsum(chunks[:-1])]) * P
    L = int(chunks.sum()) * P
    g_out = np.zeros(L, dtype=np.int64)
    oh_out = np.full(L, PAD_OH, dtype=np.float32)
    n = gidx.shape[0]
    rank = np.arange(n, dtype=np.int64) - starts[tid_s]
    dest = dest_base[tid_s] + rank
    g_out[dest] = gidx
    oh_out[dest] = ohval
    return g_out, oh_out


def build_kernel(chunks1, chunks2):
    """Build the SPMD device program.

    chunks1[g]: #chunks for edge group g (phase 1, may be 0);
    chunks2[gg]: #chunks for node group gg (phase 2, >= 1).
    """
    npc, n_node_groups, n_edge_groups, EROWS, SHARD = _derived()
    NROWS = n_node_groups * NTW
    LA = int(np.sum(chunks1)) * P
    LB = int(np.sum(chunks2)) * P
    NCA = LA // P
    NCB = LB // P
    REPS = int(os.environ.get("KREPS", "1"))
    no_gather = os.environ.get("DBG_NO_GATHER") == "1"  # timing bisection only
    no_scatmm = os.environ.get("DBG_NO_SCATMM") == "1"
    single_packet = os.environ.get("KSP", "0") == "1"

    nc = bacc.Bacc("TRN2", num_devices=NCORES)

    xT_in = nc.dram_tensor("xT", [IN_DIM, npc], BF16, kind="ExternalInput")
    w_in = nc.dram_tensor("w", [IN_DIM, OUT_DIM], BF16, kind="ExternalInput")
    biasT_in = nc.dram_tensor("biasT", [OUT_DIM, 1], F32, kind="ExternalInput")
    dinv_in = nc.dram_tensor("dinv", [1, NROWS], F32, kind="ExternalInput")
    binv_in = nc.dram_tensor("binv", [P, SHARD // P], F32, kind="ExternalInput")
    idxA_in = nc.dram_tensor("idxA", [P, LA // 16], I16, kind="ExternalInput")
    ohA_in = nc.dram_tensor("ohA", [P, NCA], BF16, kind="ExternalInput")
    idxB_in = nc.dram_tensor("idxB", [P, LB // 16], I16, kind="ExternalInput")
    ohB_in = nc.dram_tensor("ohB", [P, NCB], BF16, kind="ExternalInput")
    out_part = nc.dram_tensor("out_part", [OUT_DIM, 1], F32, kind="ExternalOutput")

    xwhl = nc.dram_tensor("xwhl", [npc, OUT_DIM], BF16)
    m_part = nc.dram_tensor("m_part", [EROWS, OUT_DIM], BF16)
    m_shard = nc.dram_tensor("m_shard", [SHARD, OUT_DIM], BF16)
    mtab_sh = nc.dram_tensor("mtab_sh", [SHARD, OUT_DIM], BF16)
    mtab = nc.dram_tensor("mtab", [EROWS, OUT_DIM], BF16, addr_space="Shared")

    # phase-1 super-groups: consecutive edge groups gathered in one call
    def make_supers(chunks):
        supers = []  # (chunk_offset, [(tile_idx, kt, local_chunk_off)])
        cur, ck, coff = [], 0, 0
        base = 0
        for t, k in enumerate(chunks):
            k = int(k)
            if k == 0:
                continue
            if ck + k > SUPER and cur:
                supers.append((coff, cur))
                coff += ck
                cur, ck = [], 0
            cur.append((t, k, ck))
            ck += k
        if cur:
            supers.append((coff, cur))
        return supers

    supers1 = make_supers(chunks1)
    supers2 = make_supers(chunks2)
    empty1 = [t for t, k in enumerate(chunks1) if int(k) == 0]

    with tile.TileContext(nc) as tc, ExitStack() as ctx:
        pin = ctx.enter_context(tc.tile_pool(name="pin", bufs=1))

        nc.gpsimd.load_library(library_config.mlp)

        # ---- persistent small tiles (once, outside reps) ----------------
        iota_i = pin.tile([P, P], I16)
        iota_bf = pin.tile([P, P], BF16)
        nc.gpsimd.iota(iota_i[:], [[1, P]], channel_multiplier=0)
        nc.vector.tensor_copy(out=iota_bf[:], in_=iota_i[:])

        def s_build(S_tile, oh_tile, col0, k, w):
            """S[p, c*w+j] = (oh[p, col0+c] == j), one DVE op for k chunks."""
            s_ap = S_tile[:].rearrange("p (k j) -> p k j", k=k)
            o = oh_tile[:, col0:col0 + k]
            in0 = bass.AP(o.tensor, o.offset, [list(o.ap[0]), list(o.ap[1]), [0, w]])
            it = iota_bf[:]
            in1 = bass.AP(it.tensor, it.offset, [list(it.ap[0]), [0, k], [1, w]])
            nc.vector.tensor_tensor(out=s_ap, in0=in0, in1=in1, op=mybir.AluOpType.is_equal)

        for rep in range(REPS):
          with tc.tile_pool(name=f"prep{rep}", bufs=1) as pr:
            # streams + per-rep persistent tiles
            idxA = pr.tile([P, LA // 16], I16, name="idxA")
            ohA = pr.tile([P, NCA], BF16, name="ohA")
            idxB = pr.tile([P, LB // 16], I16, name="idxB")
            ohB = pr.tile([P, NCB], BF16, name="ohB")
            nc.sync.dma_start(out=idxA[:], in_=idxA_in[:])
            nc.sync.dma_start(out=ohA[:], in_=ohA_in[:])
            nc.sync.dma_start(out=idxB[:], in_=idxB_in[:])
            nc.sync.dma_start(out=ohB[:], in_=ohB_in[:])
            bias_sb = pr.tile([P, 1], F32, name="biasT")
            nc.sync.dma_start(out=bias_sb[:OUT_DIM], in_=biasT_in[:])
            dinv_bc = pr.tile([P, NROWS], F32, name="dinvbc")
            nc.sync.dma_start(
                out=dinv_bc[:], in_=bass.AP(dinv_in, 0, [[0, P], [1, NROWS]]))
            binv_sb = pr.tile([P, SHARD // P], F32, name="binv")
            nc.sync.dma_start(out=binv_sb[:], in_=binv_in[:])
            accT = pr.tile([P, NROWS], F32, name="accT")

            # ---- stage A: xw = x @ W -> bf16 row table ------------------
            with tc.tile_pool(name=f"pa{rep}", bufs=1) as pa, \
                 tc.tile_pool(name=f"pa2{rep}", bufs=3) as pa2, \
                 tc.tile_pool(name=f"psa{rep}", bufs=2, space="PSUM") as psa:
                kh = IN_DIM // P
                xT_sb = [pa.tile([P, npc], BF16, name=f"xT{k}") for k in range(kh)]
                w_sb = [pa.tile([P, OUT_DIM], BF16, name=f"wsb{k}") for k in range(kh)]
                for k in range(kh):
                    nc.sync.dma_start(out=xT_sb[k][:], in_=xT_in[k * P:(k + 1) * P, :])
                    nc.sync.dma_start(out=w_sb[k][:], in_=w_in[k * P:(k + 1) * P, :])
                for i in range(0, npc, P):
                    nt = min(P, npc - i)
                    pxw = psa.tile([P, OUT_DIM], F32, tag="pxw")
                    for k in range(kh):
                        nc.tensor.matmul(
                            out=pxw[:nt], lhsT=xT_sb[k][:, i:i + nt], rhs=w_sb[k][:],
                            start=(k == 0), stop=(k == kh - 1))
                    xst = pa2.tile([P, OUT_DIM], BF16, tag="xst")
                    nc.scalar.activation(
                        out=xst[:nt], in_=pxw[:nt],
                        func=mybir.ActivationFunctionType.Copy)
                    nc.sync.dma_start(out=xwhl[i:i + nt, :], in_=xst[:nt, :])

            # ---- stage B: phase-1 scatter (node -> edge) ----------------
            with tc.tile_pool(name=f"pb{rep}", bufs=3) as pb, \
                 tc.tile_pool(name=f"psb{rep}", bufs=2, space="PSUM") as psb:
                zrow = pb.tile([P, OUT_DIM], BF16, tag="zrow", name="zrow")
                nc.vector.memset(zrow[:], 0.0)
                for t in empty1:
                    nc.sync.dma_start(
                        out=m_part[t * ETW:(t + 1) * ETW, :], in_=zrow[:ETW, :])
                for coff, groups in supers1:
                    ck_tot = sum(k for _, k, _ in groups)
                    G = pb.tile([P, ck_tot, OUT_DIM], BF16, tag="G")
                    if not no_gather:
                        nc.gpsimd.dma_gather(
                            G[:, :, :], xwhl[:, :],
                            idxA[:, coff * 8:(coff + ck_tot) * 8],
                            ck_tot * P, ck_tot * P, OUT_DIM,
                            single_packet=single_packet)
                    else:
                        nc.vector.memset(G[:, 0, :], 0.0)
                    for t, kt, loc in groups:
                        S = pb.tile([P, kt * ETW], BF16, tag="S")
                        s_build(S, ohA, coff + loc, kt, ETW)
                        pm = psb.tile([P, OUT_DIM], F32, tag="pm")
                        nkt = 1 if no_scatmm else kt
                        for c in range(nkt):
                            nc.tensor.matmul(
                                out=pm[:ETW], lhsT=S[:, c * ETW:(c + 1) * ETW],
                                rhs=G[:, loc + c, :],
                                start=(c == 0), stop=(c == nkt - 1),
                                skip_group_check=True)
                        mt = pb.tile([P, OUT_DIM], BF16, tag="mt")
                        nc.scalar.activation(
                            out=mt[:ETW], in_=pm[:ETW],
                            func=mybir.ActivationFunctionType.Copy)
                        nc.sync.dma_start(
                            out=m_part[t * ETW:(t + 1) * ETW, :], in_=mt[:ETW, :])

            # ---- stage C: ReduceScatter -> scale -> AllGather -----------
            no_cc = os.environ.get("DBG_NO_CC") == "1"  # TimelineSim can't
            if no_cc:                                   # model collectives
                nc.sync.dma_start(out=m_shard[:, :], in_=m_part[:SHARD, :])
            else:
                nc.gpsimd.collective_compute(
                    "ReduceScatter", mybir.AluOpType.add,
                    replica_groups=[list(range(NCORES))],
                    ins=[m_part[:, :]], outs=[m_shard[:, :]])
            with tc.tile_pool(name=f"pc{rep}", bufs=3) as pc:
                for t in range(SHARD // P):
                    ms = pc.tile([P, OUT_DIM], BF16, tag="ms")
                    nc.sync.dma_start(out=ms[:], in_=m_shard[t * P:(t + 1) * P, :])
                    st = pc.tile([P, OUT_DIM], BF16, tag="st")
                    nc.vector.tensor_scalar(
                        out=st[:], in0=ms[:], scalar1=binv_sb[:, t:t + 1],
                        scalar2=None, op0=mybir.AluOpType.mult)
                    nc.sync.dma_start(out=mtab_sh[t * P:(t + 1) * P, :], in_=st[:])
            if no_cc:
                for cc in range(NCORES):
                    nc.sync.dma_start(
                        out=mtab[cc * SHARD:(cc + 1) * SHARD, :],
                        in_=mtab_sh[:, :])
            else:
                nc.gpsimd.collective_compute(
                    "AllGather", mybir.AluOpType.bypass,
                    replica_groups=[list(range(NCORES))],
                    ins=[mtab_sh[:, :]], outs=[mtab[:, :]])

            # ---- stage D: phase-2 scatter (edge -> node), transposed ----
            with tc.tile_pool(name=f"pd{rep}", bufs=3) as pd, \
                 tc.tile_pool(name=f"psd{rep}", bufs=2, space="PSUM") as psd:
                for coff, groups in supers2:
                    ck_tot = sum(k for _, k, _ in groups)
                    G2 = pd.tile([P, ck_tot, OUT_DIM], BF16, tag="G2")
                    if not no_gather:
                        nc.gpsimd.dma_gather(
                            G2[:, :, :], mtab[:, :],
                            idxB[:, coff * 8:(coff + ck_tot) * 8],
                            ck_tot * P, ck_tot * P, OUT_DIM,
                            single_packet=single_packet)
                    else:
                        nc.vector.memset(G2[:, 0, :], 0.0)
                    for gg, kt, loc in groups:
                        S2 = pd.tile([P, kt * NTW], BF16, tag="S2")
                        s_build(S2, ohB, coff + loc, kt, NTW)
                        poT = psd.tile([P, NTW], F32, tag="poT")
                        nkt = 1 if no_scatmm else kt
                        for c in range(nkt):
                            nc.tensor.matmul(
                                out=poT[:], lhsT=G2[:, loc + c, :],
                                rhs=S2[:, c * NTW:(c + 1) * NTW],
                                start=(c == 0), stop=(c == nkt - 1),
                                skip_group_check=True)
                        ot = pd.tile([P, NTW], F32, tag="ot")
                        nc.vector.tensor_tensor(
                            out=ot[:], in0=poT[:],
                            in1=dinv_bc[:, gg * NTW:(gg + 1) * NTW],
                            op=mybir.AluOpType.mult)
                        nc.scalar.activation(
                            out=accT[:, gg * NTW:(gg + 1) * NTW], in_=ot[:],
                            func=mybir.ActivationFunctionType.Relu,
                            bias=bias_sb[:, 0:1])
                # zero phantom-node columns (beyond npc) before the reduce
                if NROWS > npc:
                    nc.vector.memset(accT[:, npc:NROWS], 0.0)

            # ---- stage E: row-sum over all node columns -> [OUT_DIM, 1] -
            with tc.tile_pool(name=f"pe{rep}", bufs=1) as pe:
                osum = pe.tile([P, 1], F32)
                nc.vector.tensor_reduce(
                    out=osum[:], in_=accT[:], axis=mybir.AxisListType.X,
                    op=mybir.AluOpType.add)
                nc.sync.dma_start(out=out_part[:, :], in_=osum[:OUT_DIM])

    nc.compile()
    return nc


def prepare_inputs(x, w, bias, hyperedge_index):
    """Host-side sharding: split entries by src-node shard, bucket/pad both
    phase streams, compute degrees + static chunk structure (shared by all
    cores)."""
    npc, n_node_groups, n_edge_groups, EROWS, SHARD = _derived()
    NROWS = n_node_groups * NTW
    src = np.asarray(hyperedge_index[0], dtype=np.int64)
    edge = np.asarray(hyperedge_index[1], dtype=np.int64)

    # exact degree reciprocals (host)
    deg_e = np.bincount(edge, minlength=N_EDGES).astype(np.float64)
    b_inv_full = np.zeros(EROWS, np.float32)
    nzmask = deg_e > 0
    b_inv_full[:N_EDGES][nzmask] = (1.0 / deg_e[nzmask]).astype(np.float32)

    core_of = src // npc
    per_core = []
    for c in range(NCORES):
        sel = core_of == c
        per_core.append((src[sel] - c * npc, edge[sel]))

    cnt1 = np.zeros((NCORES, n_edge_groups), np.int64)
    cnt2 = np.zeros((NCORES, n_node_groups), np.int64)
    for c, (s_loc, e_glob) in enumerate(per_core):
        cnt1[c] = np.bincount(e_glob // ETW, minlength=n_edge_groups)
        cnt2[c] = np.bincount(s_loc // NTW, minlength=n_node_groups)
    chunks1 = -(-cnt1.max(axis=0) // P)                 # may be 0
    chunks2 = np.maximum(1, -(-cnt2.max(axis=0) // P))  # >= 1 (bias/relu rows)

    in_maps = []
    for c, (s_loc, e_glob) in enumerate(per_core):
        g1, oh1 = _bucket_entries(
            s_loc, (e_glob % ETW).astype(np.float32), e_glob // ETW,
            n_edge_groups, chunks1)
        g2, oh2 = _bucket_entries(
            e_glob, (s_loc % NTW).astype(np.float32), s_loc // NTW,
            n_node_groups, chunks2)

        deg_n = np.bincount(s_loc, minlength=npc).astype(np.float64)
        d_inv = np.zeros(NROWS, np.float32)
        nz = deg_n > 0
        d_inv[:npc][nz] = (1.0 / deg_n[nz]).astype(np.float32)

        binv_shard = np.ascontiguousarray(
            b_inv_full[c * SHARD:(c + 1) * SHARD].reshape(SHARD // P, P).T)

        xT = np.ascontiguousarray(
            x[c * npc:(c + 1) * npc].T.astype(ml_dtypes.bfloat16))
        in_maps.append({
            "xT": xT,
            "w": np.ascontiguousarray(w.astype(ml_dtypes.bfloat16)),
            "biasT": np.ascontiguousarray(bias.astype(np.float32)).reshape(-1, 1),
            "dinv": d_inv.reshape(1, -1),
            "binv": binv_shard,
            "idxA": _wrap_idx16(g1),
            "ohA": _oh_cols(oh1),
            "idxB": _wrap_idx16(g2),
            "ohB": _oh_cols(oh2),
        })

    return in_maps, chunks1, chunks2


def kernel(x_node_features, lin_weight, bias, hyperedge_index):
    in_maps, chunks1, chunks2 = prepare_inputs(
        x_node_features, lin_weight, bias, hyperedge_index)
    nc = build_kernel(chunks1, chunks2)
    res = run_bass_kernel_spmd(nc, in_maps, list(range(NCORES)))
    total = np.zeros(OUT_DIM, np.float64)
    for c in range(NCORES):
        total += res.results[c]["out_part"][:, 0].astype(np.float64)
    return (total / N_NODES).astype(np.float32)
